# revision 4
# baseline (speedup 1.0000x reference)
"""Trainium2 Bass kernel for nn_Graph_Encoder (gnn_message_passing).

Strategy (8 NeuronCores, dst-sharded):
  - Host: graph preprocessing — degree norms + edge-parallel segment-sums
    fold the t-axis into 22 feature columns, giving per-edge-type dense
    messages m_i = [a_i | p_i | 1]  ([Nd, 22] per type), dst-sharded
    across the 8 cores.
  - Device: out = sum_i lrelu(v_i),  v_i = m_i @ Wt_i, decomposed as
        out = 0.01*S + 0.99*sum_i relu(v_i),   S = sum_i v_i,
    so the linear part S accumulates for free on the PE (stacked-K
    matmuls into PSUM), and each relu costs ONE engine op:
      * 7 edge types: ScalarE Relu (PSUM->SBUF bf16) + PE identity-matmul
        accumulate into the same PSUM bank as S,
      * 5 edge types: VectorE scalar_tensor_tensor (Z max 0) add acc.
    v_i matmuls are K=22, packed 4-per-PE-pass via tile_position row
    tiling. Output written bf16 (upcast on host) to halve DMA traffic.

Output: [49152, 1, 12, 128] fp32.
"""

import os
import numpy as np
import ml_dtypes

T = 12
NS = 100_000
ND = 49_152
E = 200_000
NTAB = 120_000
SH = 9
H = 128
NCORES = 8
ND_LOC = ND // NCORES          # 6144
NTILES = ND_LOC // 128         # 48
K = 22                         # 12 x-cols + 9 pe-cols + 1 const(bias) col
NF = T * H                     # 1536
NG = 3                         # free-dim groups of 512
DVE_SET = (0, 1, 4, 5, 8)      # edge types consumed by VectorE
SC_SET = tuple(i for i in range(T) if i not in DVE_SET)

_cache = {}


def _build_program():
    import concourse.bacc as bacc
    import concourse.mybir as mybir
    from concourse.tile import TileContext

    bf16 = mybir.dt.bfloat16
    f32 = mybir.dt.float32
    AT = mybir.ActivationFunctionType
    OP = mybir.AluOpType

    nc = bacc.Bacc()
    mT4_d = nc.dram_tensor("mT4", [NTILES, 128, 3 * 128], bf16, kind="ExternalInput")
    mS_d = nc.dram_tensor("mS", [NTILES, 88, 3 * 128], bf16, kind="ExternalInput")
    wt4_d = nc.dram_tensor("Wt4", [128, 3 * NF], bf16, kind="ExternalInput")
    wtS_d = nc.dram_tensor("WtS", [88, 3 * NF], bf16, kind="ExternalInput")
    eye_d = nc.dram_tensor("eye", [128, 128], bf16, kind="ExternalInput")
    out_d = nc.dram_tensor("out", [NTILES, 128, NF], bf16, kind="ExternalOutput")

    with TileContext(nc) as tc:
        with (
            tc.tile_pool(name="wt", bufs=1) as wtp,
            tc.tile_pool(name="mt", bufs=3) as mtp,
            tc.tile_pool(name="ms", bufs=3) as msp,
            tc.tile_pool(name="zp", bufs=5, space="PSUM") as zp,
            tc.tile_pool(name="pp", bufs=2, space="PSUM") as pp,
            tc.tile_pool(name="rp", bufs=6) as rp,
            tc.tile_pool(name="accp", bufs=2) as accp,
            tc.tile_pool(name="outp", bufs=3) as outp,
        ):
            wt4 = wtp.tile([128, 3 * NF], bf16, tag="wt4")
            nc.sync.dma_start(out=wt4[:], in_=wt4_d[:])
            wtS = wtp.tile([88, 3 * NF], bf16, tag="wtS")
            nc.sync.dma_start(out=wtS[:], in_=wtS_d[:])
            eye = wtp.tile([128, 128], bf16, tag="eye")
            nc.sync.dma_start(out=eye[:], in_=eye_d[:])

            for tau in range(NTILES):
                mt4 = mtp.tile([128, 3 * 128], bf16)
                nc.sync.dma_start(out=mt4[:], in_=mT4_d[tau])
                ms = msp.tile([88, 3 * 128], bf16)
                nc.sync.dma_start(out=ms[:], in_=mS_d[tau])
                accD = accp.tile([128, NF], f32)
                outt = outp.tile([128, NF], bf16)

                for g in range(NG):
                    gsl = slice(g * 512, (g + 1) * 512)
                    pacc = pp.tile([128, 512], f32, space="PSUM")
                    # S = 0.505 * sum_i v_i via 3 stacked-K (=88) matmuls
                    for c in range(3):
                        nc.tensor.matmul(
                            out=pacc[:],
                            lhsT=ms[:, c * 128:(c + 1) * 128],
                            rhs=wtS[:, c * NF + g * 512: c * NF + (g + 1) * 512],
                            start=(c == 0), stop=False,
                            skip_group_check=True,
                        )
                    n_sc_done = 0
                    for ig in range(3):
                        zs = []
                        for r in range(4):
                            z = zp.tile([128, 512], f32, space="PSUM")
                            nc.tensor.matmul(
                                out=z[:],
                                lhsT=mt4[32 * r:32 * r + K,
                                         ig * 128:(ig + 1) * 128],
                                rhs=wt4[32 * r:32 * r + K,
                                        (ig * NF) + g * 512:
                                        (ig * NF) + (g + 1) * 512],
                                start=True, stop=True,
                                tile_position=(32 * r, 0),
                            )
                            zs.append(z)
                        for r in range(4):
                            i = 4 * ig + r
                            if i in DVE_SET:
                                first = (i == DVE_SET[0])
                                nc.vector.scalar_tensor_tensor(
                                    out=accD[:, gsl],
                                    in0=zs[r][:],
                                    scalar=0.0,
                                    in1=(wt4[:, 0:512] if first
                                         else accD[:, gsl]),
                                    op0=OP.max,
                                    op1=(OP.bypass if first else OP.add),
                                )
                            else:
                                rt = rp.tile([128, 512], bf16)
                                nc.scalar.activation(
                                    out=rt[:], in_=zs[r][:], func=AT.Relu)
                                n_sc_done += 1
                                nc.tensor.matmul(
                                    out=pacc[:],
                                    lhsT=eye[:],
                                    rhs=rt[:],
                                    start=False,
                                    stop=(n_sc_done == len(SC_SET)),
                                    skip_group_check=True,
                                )
                    # out_g = accD_g + pacc   (bf16)
                    nc.vector.scalar_tensor_tensor(
                        out=outt[:, gsl],
                        in0=accD[:, gsl],
                        scalar=0.0,
                        in1=pacc[:],
                        op0=OP.bypass,
                        op1=OP.add,
                    )
                nc.sync.dma_start(out=out_d[tau], in_=outt[:])
    nc.compile()
    return nc


def _preprocess(x_src, pos_emb_src, pe_scale, emb_idx, src_idx, dst_idx, W, b):
    """Host graph preprocessing -> per-core device inputs."""
    x = np.nan_to_num(np.asarray(x_src, np.float32))[:, :, 0]       # [T, NS]
    pe = np.asarray(pos_emb_src, np.float32)[np.asarray(emb_idx)] \
        * np.asarray(pe_scale, np.float32)                          # [NS, 9]
    W = np.asarray(W, np.float32)
    b = np.asarray(b, np.float32)
    src_idx = np.asarray(src_idx)
    dst_idx = np.asarray(dst_idx)

    # feat columns: 12 x-cols then 9 pe-cols
    feat = np.concatenate([x.T, pe], axis=1)                        # [NS, 21]

    m = np.zeros((T, ND, K), np.float32)
    m[:, :, 21] = 1.0
    for i in range(T):
        s, d = src_idx[i], dst_idx[i]
        deg_s = np.bincount(s, minlength=NS).astype(np.float32)
        deg_d = np.bincount(d, minlength=ND).astype(np.float32)
        ns = np.clip(deg_s, 1.0, None) ** -0.5
        nd = np.clip(deg_d, 1.0, None) ** -0.5
        a = ns[s] * nd[d]                                           # [E]
        g = feat[s] * a[:, None]                                    # [E, 21]
        for c in range(21):
            m[i, :, c] = np.bincount(d, weights=g[:, c], minlength=ND)

    # Wt[i]: [22, T, H] -> z_{i,t} = m_i[:, t]*W[i,0] + m_pe@W[i,1:] + b
    Wt = np.zeros((T, K, T, H), np.float32)
    for t in range(T):
        Wt[:, t, t, :] = W[:, 0, :]
    Wt[:, 12:21, :, :] = W[:, 1:10, None, :]
    Wt[:, 21, :, :] = b[:, None, :]
    Wt = Wt.reshape(T, K, NF)
    WtZ = (0.99 * Wt).astype(ml_dtypes.bfloat16)
    WtS = (0.01 * Wt).astype(ml_dtypes.bfloat16)

    # Wt4 [128, 3*NF]: row 32*(i%4)+k, cols (i//4)*NF + c  -> WtZ[i][k, c]
    Wt4 = np.zeros((128, 3 * NF), ml_dtypes.bfloat16)
    # WtS88 [88, 3*NF]: row (i%4)*22+k, cols (i//4)*NF + c -> WtS[i][k, c]
    WtS88 = np.zeros((88, 3 * NF), ml_dtypes.bfloat16)
    for i in range(T):
        r, ig = i % 4, i // 4
        Wt4[32 * r:32 * r + K, ig * NF:(ig + 1) * NF] = WtZ[i]
        WtS88[22 * r:22 * r + K, ig * NF:(ig + 1) * NF] = WtS[i]

    eye = np.eye(128, dtype=ml_dtypes.bfloat16)

    in_maps = []
    for core in range(NCORES):
        sl = m[:, core * ND_LOC:(core + 1) * ND_LOC]                # [12, 6144, 22]
        # mT4 [48, 128, 384]: [tau, 32*(i%4)+k, (i//4)*128 + n] = m_i[n, k]
        mt = sl.reshape(T, NTILES, 128, K)                          # [i, tau, n, k]
        mT4 = np.zeros((NTILES, 128, 3 * 128), np.float32)
        mS = np.zeros((NTILES, 88, 3 * 128), np.float32)
        for i in range(T):
            r, ig = i % 4, i // 4
            # mt[i]: [tau, n, k] -> [tau, k, n]
            mki = mt[i].transpose(0, 2, 1)
            mT4[:, 32 * r:32 * r + K, ig * 128:(ig + 1) * 128] = mki
            # S chunks: chunk c = {4c..4c+3}; i = 4*ig + r -> chunk ig, slot r
            mS[:, 22 * r:22 * r + K, ig * 128:(ig + 1) * 128] = mki
        in_maps.append({
            "mT4": mT4.astype(ml_dtypes.bfloat16),
            "mS": mS.astype(ml_dtypes.bfloat16),
            "Wt4": Wt4, "WtS": WtS88, "eye": eye,
        })
    return in_maps


def kernel(x_src, pos_emb_src, pe_scale, emb_idx, src_idx, dst_idx, W, b):
    from concourse.bass_utils import run_bass_kernel_spmd

    in_maps = _preprocess(x_src, pos_emb_src, pe_scale, emb_idx,
                          src_idx, dst_idx, W, b)
    if "nc" not in _cache:
        _cache["nc"] = _build_program()
    nc = _cache["nc"]

    trace = bool(int(os.environ.get("KERNEL_TRACE", "0")))
    res = run_bass_kernel_spmd(nc, in_maps, core_ids=list(range(NCORES)),
                               trace=trace)
    _cache["last_results"] = res

    out = np.concatenate(
        [r["out"].reshape(ND_LOC, T, H) for r in res.results], axis=0
    ).astype(np.float32)
    return out[:, None]                                             # [ND, 1, T, H]


# revision 7
# speedup vs baseline: 1.4980x; 1.4980x over previous
"""Trainium2 Bass kernel for nn_Graph_Encoder (gnn_message_passing).

Strategy (8 NeuronCores, dst-sharded):
  - Host: graph preprocessing — degree norms + edge-parallel segment-sums
    fold the t-axis into 22 feature columns, giving per-edge-type dense
    messages m_i = [a_i | p_i | 1]  ([Nd, 22] per type), dst-sharded
    across the 8 cores.
  - Device: out = sum_i lrelu(v_i),  v_i = m_i @ Wt_i, decomposed as
        out = 0.01*S + 0.99*sum_i relu(v_i),   S = sum_i v_i,
    so the linear part S accumulates for free on the PE (stacked-K
    matmuls into PSUM), and each relu costs ONE engine op:
      * 7 edge types: ScalarE Relu (PSUM->SBUF bf16) + PE identity-matmul
        accumulate into the same PSUM bank as S,
      * 5 edge types: VectorE scalar_tensor_tensor (Z max 0) add acc.
    v_i matmuls are K=22, packed 4-per-PE-pass via tile_position row
    tiling. Output written bf16 (upcast on host) to halve DMA traffic.

Output: [49152, 1, 12, 128] fp32.
"""

import os
import numpy as np
import ml_dtypes

T = 12
NS = 100_000
ND = 49_152
E = 200_000
NTAB = 120_000
SH = 9
H = 128
NCORES = 8
ND_LOC = ND // NCORES          # 6144
NTILES = ND_LOC // 128         # 48
K = 22                         # 12 x-cols + 9 pe-cols + 1 const(bias) col
NF = T * H                     # 1536
NG = 3                         # free-dim groups of 512
DVE_SET = (0, 1, 4, 5, 8)      # edge types consumed by VectorE
SC_SET = tuple(i for i in range(T) if i not in DVE_SET)

_cache = {}


def _build_program():
    import concourse.bacc as bacc
    import concourse.mybir as mybir
    from concourse.tile import TileContext

    bf16 = mybir.dt.bfloat16
    f32 = mybir.dt.float32
    AT = mybir.ActivationFunctionType
    OP = mybir.AluOpType

    nc = bacc.Bacc()
    mT4_d = nc.dram_tensor("mT4", [NTILES, 128, 3 * 128], bf16, kind="ExternalInput")
    mS_d = nc.dram_tensor("mS", [NTILES, 88, 3 * 128], bf16, kind="ExternalInput")
    wt4_d = nc.dram_tensor("Wt4", [128, 3 * NF], bf16, kind="ExternalInput")
    wtS_d = nc.dram_tensor("WtS", [88, 3 * NF], bf16, kind="ExternalInput")
    eye_d = nc.dram_tensor("eye", [128, 128], bf16, kind="ExternalInput")
    out_d = nc.dram_tensor("out", [NTILES, 128, NF], bf16, kind="ExternalOutput")

    with TileContext(nc) as tc:
        with (
            tc.tile_pool(name="wt", bufs=1) as wtp,
            tc.tile_pool(name="mt", bufs=3) as mtp,
            tc.tile_pool(name="ms", bufs=3) as msp,
            tc.tile_pool(name="zp", bufs=5, space="PSUM") as zp,
            tc.tile_pool(name="pp", bufs=2, space="PSUM") as pp,
            tc.tile_pool(name="rp", bufs=16) as rp,
            tc.tile_pool(name="accp", bufs=2) as accp,
            tc.tile_pool(name="outp", bufs=3) as outp,
        ):
            wt4 = wtp.tile([128, 3 * NF], bf16, tag="wt4")
            nc.sync.dma_start(out=wt4[:], in_=wt4_d[:])
            wtS = wtp.tile([88, 3 * NF], bf16, tag="wtS")
            nc.sync.dma_start(out=wtS[:], in_=wtS_d[:])
            eye = wtp.tile([128, 128], bf16, tag="eye")
            nc.sync.dma_start(out=eye[:], in_=eye_d[:])

            # Software-pipelined emission: group g's identity/S burst runs on
            # the PE after group g+1's Z-packs; g's final combine runs on the
            # DVE after g+1's consumer chain — so neither engine ever stalls
            # on a cross-engine tail.
            tau_state = {}
            pending = None

            def emit_id_burst(p):
                for n_done, i in enumerate(SC_SET):
                    nc.tensor.matmul(
                        out=p["pacc"][:],
                        lhsT=eye[:],
                        rhs=p["rts"][i][:],
                        start=False,
                        stop=(n_done == len(SC_SET) - 1),
                        skip_group_check=True,
                    )

            def emit_final(p):
                nc.vector.scalar_tensor_tensor(
                    out=p["outt"][:, p["gsl"]],
                    in0=p["accD"][:, p["gsl"]],
                    scalar=0.0,
                    in1=p["pacc"][:],
                    op0=OP.bypass,
                    op1=OP.add,
                )
                if p["g"] == NG - 1:
                    nc.sync.dma_start(out=out_d[p["tau"]], in_=p["outt"][:])

            for tau in range(NTILES):
                mt4 = mtp.tile([128, 3 * 128], bf16)
                nc.sync.dma_start(out=mt4[:], in_=mT4_d[tau])
                ms = msp.tile([88, 3 * 128], bf16)
                nc.sync.dma_start(out=ms[:], in_=mS_d[tau])
                accD = accp.tile([128, NF], f32)
                outt = outp.tile([128, NF], bf16)
                tau_state[tau] = (mt4, ms, accD, outt)

                for g in range(NG):
                    gsl = slice(g * 512, (g + 1) * 512)
                    # Phase 1: all 12 Z matmuls as three 4-packs (row-tiled)
                    zs = {}
                    for ig in range(3):
                        for r in range(4):
                            z = zp.tile([128, 512], f32, space="PSUM")
                            nc.tensor.matmul(
                                out=z[:],
                                lhsT=mt4[32 * r:32 * r + K,
                                         ig * 128:(ig + 1) * 128],
                                rhs=wt4[32 * r:32 * r + K,
                                        (ig * NF) + g * 512:
                                        (ig * NF) + (g + 1) * 512],
                                start=True, stop=True,
                                tile_position=(32 * r, 0),
                            )
                            zs[4 * ig + r] = z
                    # Phase 2: S matmuls for this group (no data deps)
                    pacc = pp.tile([128, 512], f32, space="PSUM")
                    for c in range(3):
                        nc.tensor.matmul(
                            out=pacc[:],
                            lhsT=ms[:, c * 128:(c + 1) * 128],
                            rhs=wtS[:, c * NF + g * 512: c * NF + (g + 1) * 512],
                            start=(c == 0), stop=False,
                            skip_group_check=True,
                        )
                    # Deferred: previous group's identity burst (its relus are
                    # done by now), placed after this group's Z-packs on PE
                    if pending is not None:
                        emit_id_burst(pending)
                    # Phase 3: consumers — DVE chain + ScalarE relus
                    rts = {}
                    for i in range(T):
                        if i in DVE_SET:
                            first = (i == DVE_SET[0])
                            nc.vector.scalar_tensor_tensor(
                                out=accD[:, gsl],
                                in0=zs[i][:],
                                scalar=0.0,
                                in1=(wt4[:, 0:512] if first
                                     else accD[:, gsl]),
                                op0=OP.max,
                                op1=(OP.bypass if first else OP.add),
                            )
                        else:
                            rt = rp.tile([128, 512], bf16)
                            nc.scalar.activation(
                                out=rt[:], in_=zs[i][:], func=AT.Relu)
                            rts[i] = rt
                    # Deferred: previous group's final combine on DVE
                    if pending is not None:
                        emit_final(pending)
                    pending = {"pacc": pacc, "rts": rts, "accD": accD,
                               "outt": outt, "gsl": gsl, "tau": tau, "g": g}
            emit_id_burst(pending)
            emit_final(pending)
    nc.compile()
    return nc


def _preprocess(x_src, pos_emb_src, pe_scale, emb_idx, src_idx, dst_idx, W, b):
    """Host graph preprocessing -> per-core device inputs."""
    x = np.nan_to_num(np.asarray(x_src, np.float32))[:, :, 0]       # [T, NS]
    pe = np.asarray(pos_emb_src, np.float32)[np.asarray(emb_idx)] \
        * np.asarray(pe_scale, np.float32)                          # [NS, 9]
    W = np.asarray(W, np.float32)
    b = np.asarray(b, np.float32)
    src_idx = np.asarray(src_idx)
    dst_idx = np.asarray(dst_idx)

    # feat columns: 12 x-cols then 9 pe-cols
    feat = np.concatenate([x.T, pe], axis=1)                        # [NS, 21]

    m = np.zeros((T, ND, K), np.float32)
    m[:, :, 21] = 1.0
    for i in range(T):
        s, d = src_idx[i], dst_idx[i]
        deg_s = np.bincount(s, minlength=NS).astype(np.float32)
        deg_d = np.bincount(d, minlength=ND).astype(np.float32)
        ns = np.clip(deg_s, 1.0, None) ** -0.5
        nd = np.clip(deg_d, 1.0, None) ** -0.5
        a = ns[s] * nd[d]                                           # [E]
        g = feat[s] * a[:, None]                                    # [E, 21]
        for c in range(21):
            m[i, :, c] = np.bincount(d, weights=g[:, c], minlength=ND)

    # Wt[i]: [22, T, H] -> z_{i,t} = m_i[:, t]*W[i,0] + m_pe@W[i,1:] + b
    Wt = np.zeros((T, K, T, H), np.float32)
    for t in range(T):
        Wt[:, t, t, :] = W[:, 0, :]
    Wt[:, 12:21, :, :] = W[:, 1:10, None, :]
    Wt[:, 21, :, :] = b[:, None, :]
    Wt = Wt.reshape(T, K, NF)
    WtZ = (0.99 * Wt).astype(ml_dtypes.bfloat16)
    WtS = (0.01 * Wt).astype(ml_dtypes.bfloat16)

    # Wt4 [128, 3*NF]: row 32*(i%4)+k, cols (i//4)*NF + c  -> WtZ[i][k, c]
    Wt4 = np.zeros((128, 3 * NF), ml_dtypes.bfloat16)
    # WtS88 [88, 3*NF]: row (i%4)*22+k, cols (i//4)*NF + c -> WtS[i][k, c]
    WtS88 = np.zeros((88, 3 * NF), ml_dtypes.bfloat16)
    for i in range(T):
        r, ig = i % 4, i // 4
        Wt4[32 * r:32 * r + K, ig * NF:(ig + 1) * NF] = WtZ[i]
        WtS88[22 * r:22 * r + K, ig * NF:(ig + 1) * NF] = WtS[i]

    eye = np.eye(128, dtype=ml_dtypes.bfloat16)

    in_maps = []
    for core in range(NCORES):
        sl = m[:, core * ND_LOC:(core + 1) * ND_LOC]                # [12, 6144, 22]
        # mT4 [48, 128, 384]: [tau, 32*(i%4)+k, (i//4)*128 + n] = m_i[n, k]
        mt = sl.reshape(T, NTILES, 128, K)                          # [i, tau, n, k]
        mT4 = np.zeros((NTILES, 128, 3 * 128), np.float32)
        mS = np.zeros((NTILES, 88, 3 * 128), np.float32)
        for i in range(T):
            r, ig = i % 4, i // 4
            # mt[i]: [tau, n, k] -> [tau, k, n]
            mki = mt[i].transpose(0, 2, 1)
            mT4[:, 32 * r:32 * r + K, ig * 128:(ig + 1) * 128] = mki
            # S chunks: chunk c = {4c..4c+3}; i = 4*ig + r -> chunk ig, slot r
            mS[:, 22 * r:22 * r + K, ig * 128:(ig + 1) * 128] = mki
        in_maps.append({
            "mT4": mT4.astype(ml_dtypes.bfloat16),
            "mS": mS.astype(ml_dtypes.bfloat16),
            "Wt4": Wt4, "WtS": WtS88, "eye": eye,
        })
    return in_maps


def kernel(x_src, pos_emb_src, pe_scale, emb_idx, src_idx, dst_idx, W, b):
    from concourse.bass_utils import run_bass_kernel_spmd

    in_maps = _preprocess(x_src, pos_emb_src, pe_scale, emb_idx,
                          src_idx, dst_idx, W, b)
    if "nc" not in _cache:
        _cache["nc"] = _build_program()
    nc = _cache["nc"]

    trace = bool(int(os.environ.get("KERNEL_TRACE", "0")))
    res = run_bass_kernel_spmd(nc, in_maps, core_ids=list(range(NCORES)),
                               trace=trace)
    _cache["last_results"] = res

    out = np.concatenate(
        [r["out"].reshape(ND_LOC, T, H) for r in res.results], axis=0
    ).astype(np.float32)
    return out[:, None]                                             # [ND, 1, T, H]
